# revision 58
# baseline (speedup 1.0000x reference)
"""Trainium2 Bass kernel for StyleGAN2-style fused upsample(x2)+conv3x3+FIR.

Reference computation (per image):
    y1 = conv_transpose(x, w', stride=2, VALID)          # [129,129,256]
    y  = depthwise_FIR_4x4(pad(y1,1)) + b                # [128,128,256]

Implementation strategy (per NeuronCore, data-parallel over batch 16 -> 8
cores x 2 images; each image processed as 2 units of 128 output channels):

  Stage 1 (TensorE): subpixel decomposition of the stride-2 transpose conv.
    Output parity (rho,sig) of the upsampled grid is a stride-1 VALID conv
    of the zero-padded x with taps W[a,b], a = 2*di+rho.  Matmuls contract
    over in-channels (128 per chunk), fp32 data bitcast to float32r (FP22
    multiply, full PE rate at free-dim >= 256).  ScalarE interleaves the
    parity grids into a dense fp16 up-grid y1 (strided psum->SBUF copies).

  Stage 2: separable FIR as six 2-tap box passes ([1,3,3,1] = [1,1]^*3 per
    axis; the 1/16 normalisation is folded into W).  Three vertical passes
    (whole-row shifts) then three horizontal passes (1-col shifts), all
    fp16 tensor_tensor adds (2x DVE mode) distributed between VectorE and
    GpSimd by a static per-(unit,band) pattern chosen so the in-order
    queues never cross-block.  For a tuned subset of bands (PEH_BANDS) the
    horizontal FIR runs on TensorE instead (4 accumulating diagonal
    matmuls per 4-row group): emission is deferred into the next unit's
    conv chunks so the PE queue never stalls, and the last unit's bands
    form the schedule tail where PE is otherwise idle.

  Scheduling details that matter for the timeline: TensorE is p-state
  warmed with dummy matmuls while the first DMAs land; x/w arrive in
  row/ocx slices sized so conv chunk 0 starts ~6us in; each band's
  horizontal passes are emitted two bands late (software pipelining) so
  the vector queues always hold ready work; bands 3-5 of each unit are
  emitted at the next unit's start to fill the boundary bubble.

  Output is written fp16 (channel-major [n, ocx, ch, r, s]); the host
  upcasts, transposes back to NHWC and adds the bias.
"""

import sys

sys.path.insert(0, "/opt/trn_rl_repo")

import numpy as np

import concourse.bass as bass  # noqa: F401  (registers engine classes)
import concourse.mybir as mybir
import concourse.tile as tile
from concourse import bacc
from concourse.bass_utils import run_bass_kernel_spmd

F32 = mybir.dt.float32
F32R = mybir.dt.float32r
F16 = mybir.dt.float16
ADD = mybir.AluOpType.add

N_CORES = 8
IMGS_PER_CORE = 2
H = W = 64          # input spatial
UP = 129            # upsampled grid (conv_transpose output)
OUT = 128           # final spatial
C = 256             # channels
CH = 128            # channels per partition chunk
BAND = 16           # FIR band rows (8 bands per unit)
GROUP = 4           # col-FIR psum group rows (4*128 = 512 free)

XROW = W + 2          # 66: padded x row length
XFLAT = (H + 2) * XROW  # 4356: flat padded image

# (unit, band) pairs whose horizontal FIR runs on TensorE.  Emission of the
# matmuls is deferred into the next unit's conv chunks (the z tile is ready
# by then), so the in-order PE queue never stalls; the last unit's PEH bands
# are emitted at the very end, where PE is otherwise idle.
PEH_BANDS = {
    (0, 5), (0, 6), (0, 7),
    (1, 5), (1, 6), (1, 7),
    (2, 5), (2, 6), (2, 7),
    (3, 3), (3, 4), (3, 5), (3, 6), (3, 7),
}


_U3_TAIL8 = False
_ZREG = 4
_ZDEF = 7
_CPSUM = 4
_FPSUM = 4
_NWARM = 70
_OUTP = 3


def _bands(unit):
    """(band index, start row, height).  The last unit's tail is split
    into 8-row bands: its final V-chain + PE flush gate the schedule end,
    and halving the band halves that chain."""
    if unit < 3 or not _U3_TAIL8:
        return [(i, 16 * i, 16) for i in range(8)]
    return [(i, 16 * i, 16) for i in range(6)] + [
        (6 + i, 96 + 8 * i, 8) for i in range(4)
    ]
# first conv m-chunk (of 10) at which deferred PEH bands may be flushed
_FLUSH_START = 7
# how many pending PEH bands to flush inside one unit's conv
_FLUSH_PER_UNIT = 3


def _pat(unit, band):
    """Engine per FIR pass (V1,V2,V3,H1,H2,H3): 'd'=VectorE, 'p'=GpSimd.
    Once a band's pass runs on GpSimd all later passes of that band stay
    there, so VectorE's in-order queue never waits on GpSimd."""
    if (unit, band) in _PAT_OVR:
        return _PAT_OVR[(unit, band)]
    if (unit, band) in PEH_BANDS:
        return _PEH_V_PAT.get((unit, band), "ddd")
    return _H_PAT.get(band, "dddddd")


# per-band default engine patterns (overridable for tuning)
_H_PAT = {1: "dppppp", 3: "dddppp"}
_PEH_V_PAT = {}
_PAT_OVR = {}

# bands whose horizontal passes are emitted at the START of the next unit's
# section: they have no conv dependency left, so they fill the boundary
# bubble where VectorE/GpSimd would otherwise wait for the next unit's
# first psum copies
_DEFER_H = {3, 4, 5}



def _build_nc():
    nc = bacc.Bacc("TRN2", target_bir_lowering=False)

    # x arrives host-padded to 66x66 (zero border) and channel-major
    # [n, icx, ch, h*w] so each partition's DMA run is contiguous
    x_d = nc.dram_tensor("x", [IMGS_PER_CORE, 2, CH, XFLAT], F32R, kind="ExternalInput")
    # Pre-arranged conv taps: [ic_part, icx, tap(a*3+b), ocx, oc]
    w_d = nc.dram_tensor("wt", [CH, 2, 9, 2, CH], F32R, kind="ExternalInput")
    # Diagonal FIR weights, fp16: [:,0:128] = I, [:,128:256] = 3I
    d_d = nc.dram_tensor("dg", [CH, 2 * CH], F16, kind="ExternalInput")
    # channel-major fp16 output [n, ocx, ch, r, s]; host transposes to NHWC
    y_d = nc.dram_tensor("y", [IMGS_PER_CORE, 2, CH, OUT, OUT], F16, kind="ExternalOutput")

    with tile.TileContext(nc) as tc:
        with (
            tc.tile_pool(name="const", bufs=1) as constp,
            tc.tile_pool(name="xp", bufs=2) as xp,
            tc.tile_pool(name="y1p", bufs=1) as y1p,
            tc.tile_pool(name="firp", bufs=3) as firp,
            tc.tile_pool(name="zp", bufs=1) as zp,
            tc.tile_pool(name="outp", bufs=_OUTP) as outp,
            tc.tile_pool(name="cpsum", bufs=_CPSUM, space="PSUM") as cpsum,
            tc.tile_pool(name="fpsum", bufs=_FPSUM, space="PSUM") as fpsum,
        ):
            w_sb = constp.tile([CH, 2, 9, 2, CH], F32R)
            # ocx=0 taps first so unit 0's matmuls can start sooner
            nc.sync.dma_start(out=w_sb[:, :, :, 0, :], in_=w_d[:, :, :, 0, :])

            # TensorE p-state warmup: dummy matmuls on a zeroed tile while
            # the first DMAs are in flight, so the real conv starts at full
            # clock instead of paying the low-p-state penalty
            warm_sb = constp.tile([CH, CH], F16)
            nc.gpsimd.memset(warm_sb[:], 0.0)
            warm_ps = fpsum.tile([CH, GROUP, OUT], F32, tag="fps")
            for _ in range(_NWARM):
                nc.tensor.matmul(
                    warm_ps[:, 0, :],
                    lhsT=warm_sb[:],
                    rhs=warm_sb[:],
                    start=True,
                    stop=True,
                )

            def boxadd(eng, out, in0, in1):
                e = nc.vector if eng == "d" else nc.gpsimd
                e.tensor_tensor(out=out, in0=in0, in1=in1, op=ADD)

            # prefetch all images up front (fresh buffers, no WAR) so the
            # in-order SP queue never delays image 1 behind output DMAs
            x_sbs = []
            for n in range(IMGS_PER_CORE):
                # flat x image + 2 slack elems so full-row matmul spans with
                # a column offset stay in bounds (the rhs is a contiguous
                # span covering whole rows)
                x_sb = xp.tile([CH, 2, XFLAT + 2], F32R, tag="x")
                nc.vector.memset(x_sb[:, 0, XFLAT : XFLAT + 2].bitcast(F32), 0.0)
                nc.vector.memset(x_sb[:, 1, XFLAT : XFLAT + 2].bitcast(F32), 0.0)
                hx = 33 * XROW
                if n == 0:
                    # three row-chunks per icx, smallest first: chunk 0 of
                    # the conv only needs rows 0..8, so it can start ~6us in
                    for lo, hi in ((0, 10 * XROW), (10 * XROW, hx), (hx, XFLAT)):
                        for icx in range(2):
                            nc.sync.dma_start(
                                out=x_sb[:, icx, lo:hi],
                                in_=x_d[n, icx, :, lo:hi],
                            )
                    # remaining constants after unit 0's critical inputs
                    nc.sync.dma_start(
                        out=w_sb[:, :, :, 1, :], in_=w_d[:, :, :, 1, :]
                    )
                    dg_sb = constp.tile([CH, 2 * CH], F16)
                    nc.sync.dma_start(out=dg_sb[:], in_=d_d[:])
                else:
                    for lo, hi in ((0, hx), (hx, XFLAT)):
                        for icx in range(2):
                            nc.sync.dma_start(
                                out=x_sb[:, icx, lo:hi],
                                in_=x_d[n, icx, :, lo:hi],
                            )
                x_sbs.append(x_sb)

            # single y1 buffer reused by all units; the zero halo rows are
            # never overwritten, so set them once
            y1_sb = y1p.tile([CH, UP + 3, UP + 1], F16, tag="y1")
            nc.vector.memset(y1_sb[:, 0:1, 0:UP], 0.0)
            nc.vector.memset(y1_sb[:, UP + 1 : UP + 3, 0:UP], 0.0)

            # static z tiles (explicit rings): the pad columns (idx 1 and
            # UP+2) are zero forever, so memset them once instead of per band
            z_reg = [
                zp.tile([CH, BAND, UP + 3], F16, name=f"zr{i}") for i in range(_ZREG)
            ]
            z_def = [
                zp.tile([CH, BAND, UP + 3], F16, name=f"zd{i}") for i in range(_ZDEF)
            ]
            for zt in z_reg + z_def:
                nc.vector.memset(zt[:, :, 1:2], 0.0)
                nc.vector.memset(zt[:, :, UP + 2 : UP + 3], 0.0)
            z_idx = {"reg": 0, "def": 0}

            def next_z(deferred):
                ring = z_def if deferred else z_reg
                key = "def" if deferred else "reg"
                zt = ring[z_idx[key] % len(ring)]
                z_idx[key] += 1
                return zt

            # deferred PE horizontal-FIR bands: (n, ocx, r0, z tile)
            pending = []

            def flush_peh(count, fine_tail=False):
                for _ in range(min(count, len(pending))):
                    fn_, focx, fr0, fz, fbh = pending.pop(0)
                    # the very last band gets 2-row groups: each link of its
                    # matmul->copy->DMA end chain is halved (256 free is
                    # still full PE rate)
                    grp = 2 if (fine_tail and not pending) else GROUP
                    out_sb = outp.tile([CH, BAND, OUT], F16, tag="out")
                    for g0 in range(0, fbh, grp):
                        ps2 = fpsum.tile([CH, GROUP, OUT], F32, tag="fps")
                        for v in range(4):
                            dgi = 0 if v in (0, 3) else 1
                            nc.tensor.matmul(
                                ps2[:, 0:grp, :],
                                lhsT=dg_sb[:, dgi * CH : (dgi + 1) * CH],
                                rhs=fz[:, g0 : g0 + grp, v + 1 : v + 1 + OUT],
                                start=(v == 0),
                                stop=(v == 3),
                            )
                        nc.scalar.copy(
                            out=out_sb[:, g0 : g0 + grp, :],
                            in_=ps2[:, 0:grp, :],
                        )
                        # per-group DMA so the band's tail is one group, not
                        # a whole band
                        nc.sync.dma_start(
                            out=y_d[fn_, focx, :, fr0 + g0 : fr0 + g0 + grp, :],
                            in_=out_sb[:, g0 : g0 + grp, :],
                        )

            # horizontal FIR box passes for one band + output DMA
            def emit_h(pat, z, hn, hocx, hr0, bh):
                h1 = firp.tile([CH, BAND, UP + 1], F16, tag="A")
                boxadd(
                    pat[3],
                    h1[:, 0:bh, 0 : UP + 1],
                    z[:, 0:bh, 1 : UP + 2],
                    z[:, 0:bh, 2 : UP + 3],
                )
                h2 = firp.tile([CH, BAND, UP], F16, tag="B")
                boxadd(
                    pat[4],
                    h2[:, 0:bh, 0:UP],
                    h1[:, 0:bh, 0:UP],
                    h1[:, 0:bh, 1 : UP + 1],
                )
                out_sb = outp.tile([CH, BAND, OUT], F16, tag="out")
                boxadd(
                    pat[5],
                    out_sb[:, 0:bh, :],
                    h2[:, 0:bh, 0:OUT],
                    h2[:, 0:bh, 1 : OUT + 1],
                )
                nc.sync.dma_start(
                    out=y_d[hn, hocx, :, hr0 : hr0 + bh, :],
                    in_=out_sb[:, 0:bh, :],
                )

            deferred_h = []

            for n in range(IMGS_PER_CORE):
                x_sb = x_sbs[n]
                for ocx in range(2):
                    unit = 2 * n + ocx
                    # boundary-bubble filler: the previous unit's deferred
                    # horizontal bands (no conv dependency left)
                    for args in deferred_h:
                        emit_h(*args)
                    deferred_h.clear()
                    # ---------------- stage 1: conv into y1 (fp16) ----------
                    # y1_sb rows: up-row p at index p+1 (rows 0,130,131 zero)
                    # cols: up-col q at index q (col 129 pad, never read)
                    # band-major over up-row chunks: all 4 parities per
                    # m-chunk so the FIR bands can start while later rows
                    # are still being computed.  Tail chunks are 4-5 rows
                    # (not 1-2) so every matmul keeps free >= 264: fp32r
                    # below 256 free costs 4x cycles per row.
                    chunks = {
                        0: [(i * 7, 7) for i in range(8)] + [(56, 5), (61, 4)],
                        1: [(i * 7, 7) for i in range(8)] + [(56, 4), (60, 4)],
                    }
                    for ci in range(10):
                        if _FLUSH_START <= ci < _FLUSH_START + _FLUSH_PER_UNIT:
                            flush_peh(1)
                        for rho in range(2):
                            for sig in range(2):
                                nm, nn = 65 - rho, 65 - sig
                                m0, r = chunks[rho][ci]
                                dis = (0, 1) if rho == 0 else (0,)
                                djs = (0, 1) if sig == 0 else (0,)
                                ps = cpsum.tile([CH, r, XROW], F32, tag="cps")
                                # icx-major accumulation order: the first
                                # matmuls only need x[icx=0], overlapping
                                # with the x[icx=1] DMA on unit 0
                                mms = [
                                    (di, dj, icx2)
                                    for icx2 in range(2)
                                    for di in dis
                                    for dj in djs
                                ]
                                for k, (di, dj, icx2) in enumerate(mms):
                                    t = (2 * di + rho) * 3 + (2 * dj + sig)
                                    st = (m0 + 1 - di) * XROW + (1 - dj)
                                    nc.tensor.matmul(
                                        ps[:, 0:r, 0:XROW].opt({0}),
                                        lhsT=w_sb[:, icx2, t, ocx, :],
                                        rhs=x_sb[:, icx2, st : st + r * XROW],
                                        start=(k == 0),
                                        stop=(k == len(mms) - 1),
                                    )
                                # strided parity write into the up-grid
                                # (cols nn..65 of each psum row are garbage
                                # from the full-row span and are skipped)
                                nc.scalar.copy(
                                    out=y1_sb[
                                        :,
                                        1 + rho + 2 * m0 : 1 + rho + 2 * (m0 + r) : 2,
                                        sig : sig + 2 * nn : 2,
                                    ],
                                    in_=ps[:, 0:r, 0:nn],
                                )

                    # ---------------- stage 2: FIR box passes per band ------
                    # Software-pipelined emission: band k's horizontal
                    # passes are emitted after band k+2's vertical passes,
                    # so when a vertical pass is blocked on the conv
                    # frontier the in-order queues still have ready work.
                    hqueue = []
                    for bi, r0, bh in _bands(unit):
                        pe_h = (unit, bi) in PEH_BANDS
                        pat = _pat(unit, bi)
                        # vertical: z[r] = y1[r-1] + 3 y1[r] + 3 y1[r+1] + y1[r+2]
                        # (up-row p at y1 index p+1)
                        b1 = firp.tile([CH, BAND + 2, UP + 1], F16, tag="A")
                        boxadd(
                            pat[0],
                            b1[:, 0 : bh + 2, 0:UP],
                            y1_sb[:, r0 : r0 + bh + 2, 0:UP],
                            y1_sb[:, r0 + 1 : r0 + bh + 3, 0:UP],
                        )
                        b2 = firp.tile([CH, BAND + 1, UP + 1], F16, tag="B")
                        boxadd(
                            pat[1],
                            b2[:, 0 : bh + 1, 0:UP],
                            b1[:, 0 : bh + 1, 0:UP],
                            b1[:, 1 : bh + 2, 0:UP],
                        )
                        # z cols: up-col q at index q+2 (idx 1 and 131 zero)
                        z = next_z(pe_h)
                        boxadd(
                            pat[2],
                            z[:, 0:bh, 2 : UP + 2],
                            b2[:, 0:bh, 0:UP],
                            b2[:, 1 : bh + 1, 0:UP],
                        )

                        if pe_h:
                            # horizontal FIR on TensorE, deferred into the
                            # next unit's conv chunks (or the schedule tail)
                            pending.append((n, ocx, r0, z, bh))
                        elif bi in _DEFER_H and unit < 3:
                            deferred_h.append((pat, z, n, ocx, r0, bh))
                        else:
                            hqueue.append((pat, z, n, ocx, r0, bh))
                        if bi >= 2 and len(hqueue) > 0 and hqueue[0][1] is not z:
                            emit_h(*hqueue.pop(0))
                    for args in hqueue:
                        emit_h(*args)
            # schedule tail: any deferred bands, then the last unit's PE
            # horizontal-FIR bands
            for args in deferred_h:
                emit_h(*args)
            deferred_h.clear()
            flush_peh(len(pending))
    nc.compile()
    return nc


_NC_CACHE = None


def _get_nc():
    global _NC_CACHE
    if _NC_CACHE is None:
        _NC_CACHE = _build_nc()
    return _NC_CACHE


def kernel(x, w, b):
    x = np.asarray(x, dtype=np.float32)
    w = np.asarray(w, dtype=np.float32)
    b = np.asarray(b, dtype=np.float32)
    # channel-major + zero pad: [N, 2, CH, (H+2)*(W+2)]
    xt = np.zeros((x.shape[0], 2, CH, H + 2, W + 2), dtype=np.float32)
    xt[:, :, :, 1 : H + 1, 1 : W + 1] = x.transpose(0, 3, 1, 2).reshape(
        x.shape[0], 2, CH, H, W
    )
    xt = xt.reshape(x.shape[0], 2, CH, XFLAT)

    # Effective transpose-conv filter, with the separable FIR normalisation
    # (1/4 per axis) folded in.
    Wf = w[::-1, ::-1] / 16.0  # [a, b, ic, oc]
    Wr = Wf.reshape(3, 3, 2, CH, 2, CH)  # a, b, icx, ic, ocx, oc
    w_arr = np.ascontiguousarray(
        Wr.transpose(3, 2, 0, 1, 4, 5).reshape(CH, 2, 9, 2, CH)
    )
    eye = np.eye(CH, dtype=np.float16)
    dg = np.ascontiguousarray(np.concatenate([eye, 3.0 * eye], axis=1))

    in_maps = [
        {
            "x": np.ascontiguousarray(xt[IMGS_PER_CORE * c : IMGS_PER_CORE * (c + 1)]),
            "wt": w_arr,
            "dg": dg,
        }
        for c in range(N_CORES)
    ]
    nc = _get_nc()
    res = run_bass_kernel_spmd(nc, in_maps, core_ids=list(range(N_CORES)))
    # [n, 2, CH, r, s] fp16 -> [n, r, s, 2*CH] fp32 + bias
    y = np.concatenate([res.results[c]["y"] for c in range(N_CORES)], axis=0)
    y = y.reshape(-1, C, OUT, OUT).transpose(0, 2, 3, 1).astype(np.float32)
    y += b.reshape(1, 1, 1, C)
    return np.ascontiguousarray(y)


if __name__ == "__main__":
    rng = np.random.default_rng(0)
    x = rng.standard_normal((16, 64, 64, 256), dtype=np.float32)
    w = rng.standard_normal((3, 3, 256, 256), dtype=np.float32) * 0.02
    b = np.zeros((256,), dtype=np.float32)
    y = kernel(x, w, b)
    print("out:", y.shape, y.dtype)
